# revision 19
# baseline (speedup 1.0000x reference)
"""Trainium2 Bass kernel for the discrete CRPS loss.

Reference computation (per pixel = (batch, step), n=50 ensemble members):
    z_j = max(forecast_j, CLIP)
    term1 = mean_j |z_j - y|
    term2 = sum_{j,k} |z_j - z_k| / (2 n (n-1))
    out   = term1 - (1 - EPS) * term2

The O(n^2) pairwise term uses the order-statistics identity
    sum_{j,k} |z_j - z_k| = sum_{i<n} (4i - 2n + 2) z_(i)
so each pixel only needs its members sorted.  Forecasts are consumed in
fp16: the rank-weighted sum and the n=50 abs-sum of term1 tolerate the
~6e-5 relative quantization (both end up ~1e-5 relative on the output).
Sorting uses a 21-stage Batcher merge network over 64 slots on the vector
engine.  All stages are ascending — the descending half of each merge is
realized by a reversed (negative-step) access pattern in the "triangle"
stage — so every stage is exactly two tensor_tensor instructions
(min + max) with <=3 free AP dims, eligible for the DVE 2x fp16 mode.
Pad slots hold +60000, sort to the top, and merge blocks containing only
pads are pruned.  Since clip(x) = max(x, c) is monotone, sorting raw
values and clipping afterwards is exact; the clip is folded into the
weighted-sum scalar_tensor_tensor ((S max c) * W) and into the term1
difference ((x max c) - y).

Sharding: data-parallel over pixels.  64*336 = 21504 pixels -> 8 cores x
2688, each core's slice laid out as [128 partitions x 21 pixel columns],
member-major in the SBUF free dimension.  The host pre-transposes and
pre-casts each core's slice to the exact fp16 SBUF layout (including pad
slots and one dummy pixel column that pads member rows to 44B for the DVE
2x mode's 4B alignment), so the load is one contiguous 128-row DMA of
345KB — less input traffic than the raw f32 slice.
"""

import numpy as np

CLIP = -0.26787253
EPS = 1e-4
N = 50          # ensemble members
NSLOT = 64      # padded member slots for the merge network
P = 128         # SBUF partitions
PXF = 21        # pixel columns per partition
PX16 = 22       # pixel columns in fp16 tiles (+1 dummy col for 4B alignment)
PPC = P * PXF   # pixels per core = 2688
NCORES = 8
BATCH, STEPS = 64, 336
PADVAL = 60000.0

_CACHE = {}


def _stage_instrs(M):
    """Batcher merge-sort network over NSLOT member slots, all-ascending.

    Yields (in0, in1, outmin, outmax) as (base_offset, [(step, count), ...])
    per stage (one min + one max tensor_tensor).  The first stage of each
    k-merge is the "triangle" (i vs k-1-i, second half read reversed via a
    negative step); the rest are uniform ascending (m, m+s) stages.  Merge
    blocks holding only pad slots (>= N) are pruned.
    """
    k = 2
    while k <= NSLOT:
        nb = min(NSLOT // k, -(-N // k))   # blocks containing real values
        if k == 2:
            nb = NSLOT // 2  # stage 1 also copies pads into the pong buffer
        Sk = k * nb
        in0 = (0, [(k * M, nb), (1, (k // 2) * M)])
        in1 = ((k - 1) * M, [(k * M, nb), (-M, k // 2), (1, M)])
        yield in0, in1, in0, in1
        s = k // 4
        while s >= 1:
            d = [(2 * s * M, Sk // (2 * s)), (1, s * M)]
            yield (0, d), (s * M, d), (0, d), (s * M, d)
            s //= 2
        k *= 2


def _build():
    import concourse.bass as bass
    import concourse.bacc as bacc
    import concourse.mybir as mybir
    from concourse.tile import TileContext

    f32 = mybir.dt.float32
    f16 = mybir.dt.float16
    Alu = mybir.AluOpType

    nc = bacc.Bacc("TRN2", debug=False, num_devices=NCORES)

    fc16 = nc.dram_tensor("forecasts16", [P, NSLOT * PX16], f16, kind="ExternalInput")
    w16 = nc.dram_tensor("weights16", [P, NSLOT * PX16], f16, kind="ExternalInput")
    ob = nc.dram_tensor("observation", [P, PXF], f32, kind="ExternalInput")
    out_t = nc.dram_tensor("out", [P, PXF], f32, kind="ExternalOutput")

    def sub_ap(tile_ap, off, dims):
        """AP at tile_ap.offset+off with custom free [step,count] dims."""
        part = list(tile_ap.ap[0])
        free = [[st, ct] for st, ct in dims if ct != 1] or [[1, 1]]
        return bass.AP(tile_ap.tensor, tile_ap.offset + off, [part] + free)

    K2 = (1.0 - EPS) / (2.0 * N * (N - 1))  # (1-eps)/4900

    with TileContext(nc) as tc:
        with tc.tile_pool(name="pool", bufs=1) as pool:
            U16a = pool.tile([P, NSLOT * PX16], f16)  # sort ping
            U16b = pool.tile([P, NSLOT * PX16], f16)  # sort pong
            Wf = pool.tile([P, NSLOT * PX16], f16)    # rank weights 4i-98
            V = pool.tile([P, N * PX16], f32)         # weighted sorted values
            D = pool.tile([P, N * PXF], f32)          # z - y
            Y = pool.tile([P, PXF], f32)
            S1 = pool.tile([P, PXF], f32)
            Wsum = pool.tile([P, PX16], f32)
            T0 = pool.tile([P, PXF], f32)
            O = pool.tile([P, PXF], f32)

            # --- loads: the big fp16 block first on the SP ring, the rank
            #     weights behind it; the observation on the ACT ring.  Stage 1
            #     of the sort covers all 64 slots, seeding the pong buffer's
            #     pad region, so no separate pad initialization is needed.
            nc.sync.dma_start(out=U16a[:], in_=fc16.ap())
            nc.sync.dma_start(out=Wf[:], in_=w16.ap())
            nc.scalar.dma_start(out=Y[:], in_=ob.ap())

            # d = clip(x) - y, broadcast over members (Pool, fp16 -> f32)
            Zf = pool.tile([P, N * PXF], f32)
            nc.gpsimd.tensor_scalar_max(
                Zf[:].rearrange("p (m q) -> p m q", m=N),
                U16a[:].rearrange("p (m q) -> p m q", m=NSLOT)[:, :N, :PXF],
                CLIP,
            )
            y_b = bass.AP(
                Y[:].tensor, Y[:].offset, [list(Y[:].ap[0]), [0, N], [1, PXF]]
            )
            nc.gpsimd.tensor_tensor(
                D[:].rearrange("p (m q) -> p m q", m=N),
                Zf[:].rearrange("p (m q) -> p m q", m=N),
                y_b,
                op=Alu.subtract,
            )

            # --- merge-sort the 64 member slots (VectorE, fp16, 21 stages) ---
            ping, pong = U16a, U16b
            for (o0, d0), (o1, d1), (om, dm), (ox, dx) in _stage_instrs(PX16):
                i0 = sub_ap(ping[:], o0, d0)
                i1 = sub_ap(ping[:], o1, d1)
                nc.vector.tensor_tensor(sub_ap(pong[:], om, dm), i0, i1, op=Alu.min)
                nc.vector.tensor_tensor(sub_ap(pong[:], ox, dx), i0, i1, op=Alu.max)
                ping, pong = pong, ping
            S = ping  # sorted ascending; real values in slots 0..N-1

            # --- weighted rank sum over real slots, clip folded in:
            #     V = (S max CLIP) * W ;  Wsum = sum_i (4i-98) z_(i)
            nc.vector.scalar_tensor_tensor(
                V[:],
                S[:, : N * PX16],
                CLIP,
                Wf[:, : N * PX16],
                op0=Alu.max,
                op1=Alu.mult,
            )
            nc.vector.reduce_sum(
                Wsum[:],
                V[:].rearrange("p (m q) -> p q m", m=N),
                axis=mybir.AxisListType.X,
            )

            # --- term1 ---
            nc.vector.tensor_reduce(
                S1[:],
                D[:].rearrange("p (m q) -> p q m", m=N),
                axis=mybir.AxisListType.X,
                op=Alu.add,
                apply_absolute_value=True,
            )

            # --- combine: out = S1/50 - K2*Wsum ---
            nc.vector.tensor_scalar_mul(T0[:], Wsum[:, :PXF], -K2)
            nc.vector.scalar_tensor_tensor(
                O[:], S1[:], 1.0 / N, T0[:], op0=Alu.mult, op1=Alu.add
            )

            nc.sync.dma_start(out=out_t.ap(), in_=O[:])

    nc.finalize()
    return nc


def _get_nc():
    if "nc" not in _CACHE:
        _CACHE["nc"] = _build()
    return _CACHE["nc"]


def make_in_maps(forecasts: np.ndarray, observation: np.ndarray):
    fc = np.ascontiguousarray(forecasts, dtype=np.float32).reshape(
        N, NCORES, P, PXF
    )
    obs = np.ascontiguousarray(observation, dtype=np.float32).reshape(
        NCORES, P, PXF
    )

    # per-core SBUF-layout staging: [P, NSLOT, PX16] member-major fp16 with
    # +PADVAL pad slots and dummy pixel column
    fct16 = np.full((NCORES, P, NSLOT, PX16), PADVAL, dtype=np.float16)
    fct16[:, :, :N, :PXF] = np.transpose(fc, (1, 2, 0, 3))

    w = (4.0 * np.arange(NSLOT) - (2 * N - 2)).astype(np.float16)
    w16 = np.broadcast_to(
        np.repeat(w, PX16).reshape(1, NSLOT * PX16), (P, NSLOT * PX16)
    )
    w16 = np.ascontiguousarray(w16, dtype=np.float16)

    return [
        {
            "forecasts16": fct16[c].reshape(P, NSLOT * PX16),
            "weights16": w16,
            "observation": obs[c],
        }
        for c in range(NCORES)
    ]


def kernel(forecasts: np.ndarray, observation: np.ndarray) -> np.ndarray:
    from concourse.bass_utils import run_bass_kernel_spmd

    in_maps = make_in_maps(forecasts, observation)
    res = run_bass_kernel_spmd(_get_nc(), in_maps, core_ids=list(range(NCORES)))
    out = np.concatenate([r["out"].reshape(PPC) for r in res.results])
    return out.reshape(BATCH, STEPS)
